# revision 1
# baseline (speedup 1.0000x reference)
"""Trainium2 Bass kernel for a 2-layer GraphConv GNN + mean-pool + linear.

Reference computation (all fp32):
    h1 = leaky_relu(segsum(w*x[src] -> dst) @ W1_rel + x @ W1_root + b1)
    h2 = leaky_relu(segsum(w*h1[src] -> dst) @ W2_rel + h1 @ W2_root + b2)
    pooled = segment_mean(h2, batch, 512)
    out = pooled @ Wl_root + bl            # [512, 8]

Distribution (8 NeuronCores):
    - Nodes sharded in contiguous ranges of 12500 per core; edges assigned to
      the core owning their dst node (host sorts edges by dst).
    - Each core gathers x[src] / h1[src] rows (256B each) from its local HBM
      copy via indirect DMA; h1 shards are exchanged with an AllGather.
    - Scatter-add to dst is a one-hot matmul: for each chunk of 128 edges
      (sorted by dst, grouped into 128-dst-node blocks) DVE builds
      onehot[e, s] = (s == dst_in_block[e]) * w[e] and TensorE contracts
      lhsT=gathered_rows[128e, 64f] x rhs=onehot[128e, 128s] into a
      feature-major PSUM tile agg[64f, 128s].
    - z (feature-major) = W_cat^T @ [agg; x_fm] in one matmul; ScalarE applies
      bias + leaky-relu; TensorE transposes back to node-major for the h1
      store / pooling.
    - Per-graph pooling is another one-hot matmul accumulated over all blocks
      into one PSUM bank; per-core partial pooled sums are returned and the
      trivial overlap-add + mean + final 64x8 linear run on host.
"""

import math
import os

import numpy as np

import concourse.bacc as bacc
import concourse.bass as bass
import concourse.mybir as mybir
import concourse.tile as tile
from concourse.bass_utils import run_bass_kernel_spmd

F32 = mybir.dt.float32
I32 = mybir.dt.int32
ALU = mybir.AluOpType
ACTF = mybir.ActivationFunctionType


class Cfg:
    def __init__(self, n_nodes, n_edges, d, n_graphs, n_cores=8, block=128):
        assert n_nodes % n_cores == 0
        self.N = n_nodes
        self.E = n_edges
        self.D = d
        self.G = n_graphs
        self.CORES = n_cores
        self.BLOCK = block
        self.NPC = n_nodes // n_cores                    # nodes per core
        self.NB = math.ceil(self.NPC / block)            # blocks per core
        self.NPAD = self.NB * block                      # padded nodes/core
        self.LEAKY = 0.01


REAL_CFG = Cfg(100000, 1250000, 64, 512)


# ---------------------------------------------------------------------------
# Host-side preprocessing: shard + sort edges, build padded per-core arrays.
# ---------------------------------------------------------------------------

def preprocess(cfg, x, edge_index, weights, batch):
    N, E, D, CORES = cfg.N, cfg.E, cfg.D, cfg.CORES
    NPC, NB, NPAD, BLOCK = cfg.NPC, cfg.NB, cfg.NPAD, cfg.BLOCK

    src = np.asarray(edge_index[0], dtype=np.int64)
    dst = np.asarray(edge_index[1], dtype=np.int64)
    w = np.asarray(weights, dtype=np.float32)
    batch = np.asarray(batch, dtype=np.int64)

    order = np.argsort(dst, kind="stable")
    src_s, dst_s, w_s = src[order], dst[order], w[order]

    core_of = dst_s // NPC
    ld = dst_s - core_of * NPC
    blk = ld // BLOCK
    dib = (ld - blk * BLOCK).astype(np.float32)          # dst-in-block

    gb = core_of * NB + blk                              # global block id
    counts = np.bincount(gb, minlength=CORES * NB).reshape(CORES, NB)
    # chunks per block: shared across cores (SPMD => identical program)
    K_list = np.maximum(1, -(-counts.max(axis=0) // 128)).astype(np.int64)
    col_start = np.concatenate([[0], np.cumsum(K_list)])
    C = int(col_start[-1])                               # total chunk columns

    # position of each edge inside the padded per-core stream
    first_in_block = np.concatenate([[0], np.cumsum(counts.reshape(-1))])[:-1]
    rank = np.arange(E, dtype=np.int64) - first_in_block[gb]
    slot = col_start[blk] * 128 + rank                   # within-core slot

    src2_val = (src_s // NPC) * NPAD + (src_s % NPC)     # padded-layout id

    src1_pad = np.zeros((CORES, C * 128), dtype=np.int32)
    src2_pad = np.zeros((CORES, C * 128), dtype=np.int32)
    dib_pad = np.full((CORES, C * 128), -1.0, dtype=np.float32)
    w_pad = np.zeros((CORES, C * 128), dtype=np.float32)
    src1_pad[core_of, slot] = src_s.astype(np.int32)
    src2_pad[core_of, slot] = src2_val.astype(np.int32)
    dib_pad[core_of, slot] = dib
    w_pad[core_of, slot] = w_s

    # [128, C] layout: column j = chunk j, partition p = edge j*128+p
    def tocol(a):
        return np.ascontiguousarray(a.reshape(C, 128).T)

    g_base = batch[np.arange(CORES) * NPC]
    in_maps = []
    for c in range(CORES):
        xs = x[c * NPC:(c + 1) * NPC]
        xT = np.zeros((D, NPAD), dtype=np.float32)
        xT[:, :NPC] = xs.T
        gs = np.full(NPAD, -1.0, dtype=np.float32)
        gs[:NPC] = (batch[c * NPC:(c + 1) * NPC] - g_base[c]).astype(np.float32)
        assert gs.max() < 128.0, "graph span per core exceeds 128"
        in_maps.append({
            "x": np.ascontiguousarray(x),
            "xT": xT,
            "src1": tocol(src1_pad[c]),
            "src2": tocol(src2_pad[c]),
            "dib": tocol(dib_pad[c]),
            "wgt": tocol(w_pad[c]),
            "gslot": np.ascontiguousarray(gs.reshape(NB, 128).T),
        })
    return in_maps, K_list.tolist(), col_start.tolist(), g_base


# ---------------------------------------------------------------------------
# Bass program
# ---------------------------------------------------------------------------

def build_nc(cfg, K_list, col_start, weights_np):
    """weights_np: dict with W1c [128,64], W2c [128,64], b1 [64,1], b2 [64,1],
    iota [128,128], id64 [64,64] (identical on every core -> baked as consts is
    not supported, passed as inputs instead)."""
    N, D, CORES = cfg.N, cfg.D, cfg.CORES
    NB, NPAD = cfg.NB, cfg.NPAD
    C = col_start[-1]
    K_MAX = max(K_list)

    nc = bacc.Bacc("TRN2", target_bir_lowering=False, debug=False,
                   num_devices=CORES)

    x_d = nc.dram_tensor("x", [N, D], F32, kind="ExternalInput")
    xT_d = nc.dram_tensor("xT", [D, NPAD], F32, kind="ExternalInput")
    src1_d = nc.dram_tensor("src1", [128, C], I32, kind="ExternalInput")
    src2_d = nc.dram_tensor("src2", [128, C], I32, kind="ExternalInput")
    dib_d = nc.dram_tensor("dib", [128, C], F32, kind="ExternalInput")
    wgt_d = nc.dram_tensor("wgt", [128, C], F32, kind="ExternalInput")
    gslot_d = nc.dram_tensor("gslot", [128, NB], F32, kind="ExternalInput")
    w1c_d = nc.dram_tensor("W1c", [2 * D, D], F32, kind="ExternalInput")
    w2c_d = nc.dram_tensor("W2c", [2 * D, D], F32, kind="ExternalInput")
    b1_d = nc.dram_tensor("b1", [D, 1], F32, kind="ExternalInput")
    b2_d = nc.dram_tensor("b2", [D, 1], F32, kind="ExternalInput")
    iota_d = nc.dram_tensor("iota", [128, 128], F32, kind="ExternalInput")
    id64_d = nc.dram_tensor("id64", [D, D], F32, kind="ExternalInput")

    pool_d = nc.dram_tensor("pool", [128, D], F32, kind="ExternalOutput")

    h1_local = nc.dram_tensor("h1_local", [NPAD, D], F32)
    h1_full = nc.dram_tensor("h1_full", [NPAD * CORES, D], F32,
                             addr_space="Shared")

    with tile.TileContext(nc) as tc:
        with (
            tc.tile_pool(name="persist", bufs=1) as pp,
            tc.tile_pool(name="work", bufs=4) as wp,
            tc.tile_pool(name="gat", bufs=12) as gp,
            tc.tile_pool(name="ps", bufs=2, space="PSUM") as psp,
            tc.tile_pool(name="pool1", bufs=1, space="PSUM") as pool1,
        ):
            # ---- persistent tiles -------------------------------------
            xT_s = pp.tile([D, NPAD], F32, tag="xT")
            h1T_s = pp.tile([D, NPAD], F32, tag="h1T")
            src1_s = pp.tile([128, C], I32, tag="src1")
            src2_s = pp.tile([128, C], I32, tag="src2")
            dib_s = pp.tile([128, C], F32, tag="dib")
            wgt_s = pp.tile([128, C], F32, tag="wgt")
            gslot_s = pp.tile([128, NB], F32, tag="gslot")
            w1c_s = pp.tile([2 * D, D], F32, tag="w1c")
            w2c_s = pp.tile([2 * D, D], F32, tag="w2c")
            b1_s = pp.tile([D, 1], F32, tag="b1")
            b2_s = pp.tile([D, 1], F32, tag="b2")
            iota_s = pp.tile([128, 128], F32, tag="iota")
            id64_s = pp.tile([D, D], F32, tag="id64")

            for t, d in [(xT_s, xT_d), (src1_s, src1_d), (src2_s, src2_d),
                         (dib_s, dib_d), (wgt_s, wgt_d), (gslot_s, gslot_d),
                         (w1c_s, w1c_d), (w2c_s, w2c_d), (b1_s, b1_d),
                         (b2_s, b2_d), (iota_s, iota_d), (id64_s, id64_d)]:
                nc.sync.dma_start(out=t[:], in_=d[:, :])

            pool_ps = pool1.tile([128, D], F32, tag="pool")

            def layer(src_s, table_ap, wc_s, b_s, xfm_s, last_stage):
                """one GraphConv layer over all blocks.
                last_stage(b, h_fm_slice_ap): consume the block's fm output."""
                for b in range(NB):
                    kb = K_list[b]
                    c0 = col_start[b]
                    agg_ps = psp.tile([D, 128], F32, tag="agg")
                    for k in range(kb):
                        # NOTE: HW generates one descriptor per dest
                        # partition-row and consumes one index per row, so
                        # each indirect gather moves exactly 128 rows.
                        gbuf = gp.tile([128, D], F32, tag="gbuf")
                        nc.gpsimd.indirect_dma_start(
                            out=gbuf[:],
                            out_offset=None,
                            in_=table_ap,
                            in_offset=bass.IndirectOffsetOnAxis(
                                ap=src_s[:, c0 + k:c0 + k + 1], axis=0),
                        )
                        oh = wp.tile([128, 128], F32, tag="oh")
                        nc.vector.tensor_scalar(
                            out=oh[:], in0=iota_s[:],
                            scalar1=dib_s[:, c0 + k:c0 + k + 1],
                            scalar2=wgt_s[:, c0 + k:c0 + k + 1],
                            op0=ALU.is_equal, op1=ALU.mult)
                        nc.tensor.matmul(
                            out=agg_ps[:],
                            lhsT=gbuf[:],
                            rhs=oh[:],
                            start=(k == 0), stop=(k == kb - 1))
                    cat = wp.tile([2 * D, 128], F32, tag="cat")
                    nc.scalar.activation(out=cat[0:D, :], in_=agg_ps[:],
                                         func=ACTF.Copy)
                    nc.scalar.activation(out=cat[D:2 * D, :],
                                         in_=xfm_s[:, b * 128:(b + 1) * 128],
                                         func=ACTF.Copy)
                    z_ps = psp.tile([D, 128], F32, tag="z")
                    nc.tensor.matmul(out=z_ps[:], lhsT=wc_s[:], rhs=cat[:],
                                     start=True, stop=True)
                    last_stage(b, z_ps, b_s)

            def leaky(dst_ap, z_ps, b_s):
                """dst = leaky_relu(z + b); sim-supported ops only."""
                zb = wp.tile([D, 128], F32, tag="zb")
                nc.scalar.activation(out=zb[:], in_=z_ps[:],
                                     func=ACTF.Identity, bias=b_s[:, 0:1])
                t = wp.tile([D, 128], F32, tag="zt")
                nc.vector.tensor_scalar_mul(out=t[:], in0=zb[:],
                                            scalar1=cfg.LEAKY)
                nc.vector.tensor_tensor(out=dst_ap, in0=zb[:], in1=t[:],
                                        op=ALU.max)

            # ---------------- layer 1 ----------------
            def l1_tail(b, z_ps, b_s):
                hslice = h1T_s[:, b * 128:(b + 1) * 128]
                leaky(hslice, z_ps, b1_s)
                t_ps = psp.tile([128, D], F32, tag="tp")
                nc.tensor.transpose(out=t_ps[:], in_=hslice, identity=id64_s[:])
                h1nm = wp.tile([128, D], F32, tag="h1nm")
                nc.vector.tensor_copy(out=h1nm[:], in_=t_ps[:])
                nc.sync.dma_start(out=h1_local[b * 128:(b + 1) * 128, :],
                                  in_=h1nm[:])

            layer(src1_s, x_d[:, :], w1c_s, b1_s, xT_s, l1_tail)

            nc.gpsimd.collective_compute(
                "AllGather",
                ALU.bypass,
                replica_groups=[list(range(CORES))],
                ins=[h1_local.ap()],
                outs=[h1_full.ap()],
            )

            # ---------------- layer 2 ----------------
            def l2_tail(b, z_ps, b_s):
                h2fm = wp.tile([D, 128], F32, tag="h2fm")
                leaky(h2fm[:], z_ps, b2_s)
                t_ps = psp.tile([128, D], F32, tag="tp")
                nc.tensor.transpose(out=t_ps[:], in_=h2fm[:],
                                    identity=id64_s[:])
                h2nm = wp.tile([128, D], F32, tag="h2nm")
                nc.vector.tensor_copy(out=h2nm[:], in_=t_ps[:])
                ph = wp.tile([128, 128], F32, tag="ph")
                nc.vector.tensor_scalar(
                    out=ph[:], in0=iota_s[:],
                    scalar1=gslot_s[:, b:b + 1], scalar2=None,
                    op0=ALU.is_equal)
                nc.tensor.matmul(out=pool_ps[:], lhsT=ph[:], rhs=h2nm[:],
                                 start=(b == 0), stop=(b == NB - 1))

            layer(src2_s, h1_full.ap(), w2c_s, b2_s, h1T_s, l2_tail)

            pool_s = wp.tile([128, D], F32, tag="pools")
            nc.scalar.activation(out=pool_s[:], in_=pool_ps[:], func=ACTF.Copy)
            nc.sync.dma_start(out=pool_d[:, :], in_=pool_s[:])

    nc.compile()
    return nc


# ---------------------------------------------------------------------------
# Entry point
# ---------------------------------------------------------------------------

_CACHE = {}


def _common_inputs(cfg, W1_root, W1_rel, W2_root, W2_rel, b1, b2):
    D = cfg.D
    return {
        "W1c": np.concatenate([W1_rel, W1_root], axis=0).astype(np.float32),
        "W2c": np.concatenate([W2_rel, W2_root], axis=0).astype(np.float32),
        "b1": np.ascontiguousarray(b1.reshape(D, 1).astype(np.float32)),
        "b2": np.ascontiguousarray(b2.reshape(D, 1).astype(np.float32)),
        "iota": np.broadcast_to(np.arange(128, dtype=np.float32),
                                (128, 128)).copy(),
        "id64": np.eye(D, dtype=np.float32),
    }


def run(cfg, inputs, trace=False):
    x = np.asarray(inputs["x_embeddings"], dtype=np.float32)
    in_maps, K_list, col_start, g_base = preprocess(
        cfg, x, inputs["edge_index"], inputs["weights"], inputs["batch"])
    common = _common_inputs(cfg, inputs["W1_root"], inputs["W1_rel"],
                            inputs["W2_root"], inputs["W2_rel"],
                            inputs["b1"], inputs["b2"])
    for m in in_maps:
        m.update(common)

    key = (cfg.N, cfg.E, tuple(K_list))
    if key not in _CACHE:
        _CACHE[key] = build_nc(cfg, K_list, col_start, common)
    nc = _CACHE[key]

    res = run_bass_kernel_spmd(nc, in_maps, core_ids=list(range(cfg.CORES)),
                               trace=trace)

    # host-side finish: overlap-add partial pooled sums, mean, final linear
    batch = np.asarray(inputs["batch"], dtype=np.int64)
    counts = np.bincount(batch, minlength=cfg.G).astype(np.float32)
    pooled = np.zeros((cfg.G + 128, cfg.D), dtype=np.float32)
    for c in range(cfg.CORES):
        pooled[g_base[c]:g_base[c] + 128] += res.results[c]["pool"]
    pooled = pooled[:cfg.G] / np.maximum(counts, 1.0)[:, None]
    out = pooled @ np.asarray(inputs["Wl_root"], dtype=np.float32)
    out = out + np.asarray(inputs["bl"], dtype=np.float32)
    return out.astype(np.float32), res


def kernel(**inputs) -> np.ndarray:
    out, _ = run(REAL_CFG, inputs, trace=False)
    return out



# revision 2
# speedup vs baseline: 1.3364x; 1.3364x over previous
"""Trainium2 Bass kernel for a 2-layer GraphConv GNN + mean-pool + linear.

Reference computation (all fp32):
    h1 = leaky_relu(segsum(w*x[src] -> dst) @ W1_rel + x @ W1_root + b1)
    h2 = leaky_relu(segsum(w*h1[src] -> dst) @ W2_rel + h1 @ W2_root + b2)
    pooled = segment_mean(h2, batch, 512)
    out = pooled @ Wl_root + bl            # [512, 8]

This version replaces per-chunk indirect_dma_start gathers (~0.9us of serial Pool/Q7
descriptor generation per 128 edges) with batched dma_gather custom-ucode
calls (~2.4us per 512 edges), cutting the dominant gather cost ~2x.

dma_gather constraints and how they're met:
  - elem_size_bytes % 256 == 0  -> fp32 rows of 64 features (256B).
  - int16 indices (< 32768)     -> gather through 4 strided table views
    (elem_step=256 elems = 4 rows, base offset r rows); idx = src//4 with
    edges grouped per dst-block by residue r = src%4. Works for both tables
    since NPC=12500 and NPAD=12544 are divisible by 4 (so src%4 residues are
    preserved in the padded h1 layout).
  - indices wrapped [i%16, i//16] into 16 partitions, replicated 8x down.

Distribution (8 NeuronCores), as v1: nodes in contiguous ranges of 12500 per
core; edges on the dst-owning core; scatter-add via one-hot matmuls; h1
exchanged with an AllGather; per-graph pooling via one-hot matmul; trivial
overlap-add + mean + final 64x8 linear on host.
"""

import math

import numpy as np

import concourse.bacc as bacc
import concourse.bass as bass
import concourse.mybir as mybir
import concourse.tile as tile
from concourse.bass_utils import run_bass_kernel_spmd

F32 = mybir.dt.float32
I16 = mybir.dt.int16
ALU = mybir.AluOpType
ACTF = mybir.ActivationFunctionType

NRES = 4  # residue groups (table views); int16 idx limit / elem_step=4 rows


class Cfg:
    def __init__(self, n_nodes, n_edges, d, n_graphs, n_cores=8, block=128):
        assert n_nodes % n_cores == 0
        self.N = n_nodes
        self.E = n_edges
        self.D = d
        self.G = n_graphs
        self.CORES = n_cores
        self.BLOCK = block
        self.NPC = n_nodes // n_cores                    # nodes per core
        self.NB = math.ceil(self.NPC / block)            # blocks per core
        self.NPAD = self.NB * block                      # padded nodes/core
        self.LEAKY = 0.01
        assert self.NPC % NRES == 0 and self.NPAD % NRES == 0


REAL_CFG = Cfg(100000, 1250000, 64, 512)


# ---------------------------------------------------------------------------
# Host-side preprocessing: shard edges by dst core, group per (dst-block,
# src%4), pad each group to chunks of 128, build int16 gather-index streams.
# ---------------------------------------------------------------------------

def _wrap16(idx):
    """[n] int -> [128, n//16] int16: i at [i%16, i//16], replicated 8x."""
    n = len(idx)
    w = np.ascontiguousarray(idx.reshape(n // 16, 16).T).astype(np.int16)
    return np.tile(w, (8, 1))


def preprocess(cfg, x, edge_index, weights, batch):
    N, E, D, CORES = cfg.N, cfg.E, cfg.D, cfg.CORES
    NPC, NB, NPAD, BLOCK = cfg.NPC, cfg.NB, cfg.NPAD, cfg.BLOCK

    src = np.asarray(edge_index[0], dtype=np.int64)
    dst = np.asarray(edge_index[1], dtype=np.int64)
    w = np.asarray(weights, dtype=np.float32)
    batch = np.asarray(batch, dtype=np.int64)

    core_of = dst // NPC
    ld = dst - core_of * NPC
    blk = ld // BLOCK
    dib = ld - blk * BLOCK
    res = src % NRES
    # sort edges by (core, block, residue); within-group order irrelevant
    order = np.lexsort((res, blk, core_of))
    src_s, w_s = src[order], w[order]
    core_s, blk_s, res_s, dib_s = (core_of[order], blk[order], res[order],
                                   dib[order].astype(np.float32))

    gid = (core_s * NB + blk_s) * NRES + res_s          # group id
    NG = CORES * NB * NRES
    counts = np.bincount(gid, minlength=NG).reshape(CORES, NB * NRES)
    # chunks per (block, residue): max over cores (SPMD identical program)
    K_br = np.maximum(1, -(-counts.max(axis=0) // BLOCK))  # [NB*NRES]
    cs = np.concatenate([[0], np.cumsum(K_br)])         # chunk col offsets
    C = int(cs[-1])                                     # chunks per core

    first = np.concatenate([[0], np.cumsum(counts.reshape(-1))])[:-1]
    rank = np.arange(E, dtype=np.int64) - first[gid]
    slot = cs[(blk_s * NRES + res_s)] * BLOCK + rank    # slot within core

    # per-core padded slot arrays
    q1 = np.zeros((CORES, C * BLOCK), dtype=np.int16)
    q2 = np.zeros((CORES, C * BLOCK), dtype=np.int16)
    dibp = np.full((CORES, C * BLOCK), -1.0, dtype=np.float32)
    wp = np.zeros((CORES, C * BLOCK), dtype=np.float32)
    q1[core_s, slot] = (src_s // NRES).astype(np.int16)
    sp = (src_s // NPC) * NPAD + (src_s % NPC)          # padded-layout id
    q2[core_s, slot] = (sp // NRES).astype(np.int16)
    dibp[core_s, slot] = dib_s
    wp[core_s, slot] = w_s

    def tocol(a):                       # [C*128] -> [128, C] (col per chunk)
        return np.ascontiguousarray(a.reshape(C, BLOCK).T)

    g_base = batch[np.arange(CORES) * NPC]
    in_maps = []
    for c in range(CORES):
        xs = x[c * NPC:(c + 1) * NPC]
        xT = np.zeros((D, NPAD), dtype=np.float32)
        xT[:, :NPC] = xs.T
        gs = np.full(NPAD, -1.0, dtype=np.float32)
        gs[:NPC] = (batch[c * NPC:(c + 1) * NPC] - g_base[c]).astype(
            np.float32)
        assert gs.max() < 128.0, "graph span per core exceeds 128"
        in_maps.append({
            "x": np.ascontiguousarray(x),
            "xT": xT,
            "idx1": _wrap16(q1[c]),
            "idx2": _wrap16(q2[c]),
            "dib": tocol(dibp[c]),
            "wgt": tocol(wp[c]),
            "gslot": np.ascontiguousarray(gs.reshape(NB, BLOCK).T),
        })
    return in_maps, K_br.reshape(NB, NRES).tolist(), cs.tolist(), g_base


# ---------------------------------------------------------------------------
# Bass program
# ---------------------------------------------------------------------------

def build_nc(cfg, K_br, cs):
    """K_br: [NB][NRES] chunks per (block, residue); cs: chunk col offsets
    (len NB*NRES+1)."""
    N, D, CORES = cfg.N, cfg.D, cfg.CORES
    NB, NPAD = cfg.NB, cfg.NPAD
    C = cs[-1]

    nc = bacc.Bacc("TRN2", target_bir_lowering=False, debug=False,
                   num_devices=CORES)

    x_d = nc.dram_tensor("x", [N, D], F32, kind="ExternalInput")
    xT_d = nc.dram_tensor("xT", [D, NPAD], F32, kind="ExternalInput")
    idx1_d = nc.dram_tensor("idx1", [128, C * 8], I16, kind="ExternalInput")
    idx2_d = nc.dram_tensor("idx2", [128, C * 8], I16, kind="ExternalInput")
    dib_d = nc.dram_tensor("dib", [128, C], F32, kind="ExternalInput")
    wgt_d = nc.dram_tensor("wgt", [128, C], F32, kind="ExternalInput")
    gslot_d = nc.dram_tensor("gslot", [128, NB], F32, kind="ExternalInput")
    w1c_d = nc.dram_tensor("W1c", [2 * D, D], F32, kind="ExternalInput")
    w2c_d = nc.dram_tensor("W2c", [2 * D, D], F32, kind="ExternalInput")
    b1_d = nc.dram_tensor("b1", [D, 1], F32, kind="ExternalInput")
    b2_d = nc.dram_tensor("b2", [D, 1], F32, kind="ExternalInput")
    iota_d = nc.dram_tensor("iota", [128, 128], F32, kind="ExternalInput")
    id64_d = nc.dram_tensor("id64", [D, D], F32, kind="ExternalInput")

    pool_d = nc.dram_tensor("pool", [128, D], F32, kind="ExternalOutput")

    h1_local = nc.dram_tensor("h1_local", [NPAD, D], F32)
    h1_full = nc.dram_tensor("h1_full", [NPAD * CORES, D], F32,
                             addr_space="Shared")

    with tile.TileContext(nc) as tc:
        with (
            tc.tile_pool(name="persist", bufs=1) as pp,
            tc.tile_pool(name="work", bufs=4) as wp,
            tc.tile_pool(name="gat", bufs=6) as gp,
            tc.tile_pool(name="ps", bufs=2, space="PSUM") as psp,
            tc.tile_pool(name="pool1", bufs=1, space="PSUM") as pool1,
        ):
            xT_s = pp.tile([D, NPAD], F32, tag="xT")
            h1T_s = pp.tile([D, NPAD], F32, tag="h1T")
            idx1_s = pp.tile([128, C * 8], I16, tag="idx1")
            idx2_s = pp.tile([128, C * 8], I16, tag="idx2")
            dib_s = pp.tile([128, C], F32, tag="dib")
            wgt_s = pp.tile([128, C], F32, tag="wgt")
            gslot_s = pp.tile([128, NB], F32, tag="gslot")
            w1c_s = pp.tile([2 * D, D], F32, tag="w1c")
            w2c_s = pp.tile([2 * D, D], F32, tag="w2c")
            b1_s = pp.tile([D, 1], F32, tag="b1")
            b2_s = pp.tile([D, 1], F32, tag="b2")
            iota_s = pp.tile([128, 128], F32, tag="iota")
            id64_s = pp.tile([D, D], F32, tag="id64")

            for t, d in [(xT_s, xT_d), (idx1_s, idx1_d), (idx2_s, idx2_d),
                         (dib_s, dib_d), (wgt_s, wgt_d), (gslot_s, gslot_d),
                         (w1c_s, w1c_d), (w2c_s, w2c_d), (b1_s, b1_d),
                         (b2_s, b2_d), (iota_s, iota_d), (id64_s, id64_d)]:
                nc.sync.dma_start(out=t[:], in_=d[:, :])

            pool_ps = pool1.tile([128, D], F32, tag="pool")

            def layer(idx_s, table_d, nrows, wc_s, b_s, xfm_s, last_stage):
                # strided views: row stride 4 rows (256 f32), base offset r
                tabv = table_d[:, :].rearrange("(a b) f -> a (b f)", b=NRES)
                for b in range(NB):
                    agg_ps = psp.tile([D, 128], F32, tag="agg")
                    total = sum(K_br[b])
                    done = 0
                    for r in range(NRES):
                        kb = K_br[b][r]
                        gi = b * NRES + r
                        c0 = cs[gi]
                        g = gp.tile([128, kb * 64], F32, tag="g")
                        gv = g[:].rearrange("p (c f) -> p c f", c=kb)
                        nc.gpsimd.dma_gather(
                            out_ap=gv,
                            in_ap=tabv[:, r * 64:(r + 1) * 64],
                            idxs_ap=idx_s[:, c0 * 8:(c0 + kb) * 8],
                            num_idxs=kb * 128, num_idxs_reg=kb * 128,
                            elem_size=64, elem_step=NRES * 64)
                        for j in range(kb):
                            cc = c0 + j
                            oh = wp.tile([128, 128], F32, tag="oh")
                            nc.vector.tensor_scalar(
                                out=oh[:], in0=iota_s[:],
                                scalar1=dib_s[:, cc:cc + 1],
                                scalar2=wgt_s[:, cc:cc + 1],
                                op0=ALU.is_equal, op1=ALU.mult)
                            nc.tensor.matmul(
                                out=agg_ps[:],
                                lhsT=g[:, j * 64:(j + 1) * 64],
                                rhs=oh[:],
                                start=(done == 0), stop=(done == total - 1))
                            done += 1
                    cat = wp.tile([2 * D, 128], F32, tag="cat")
                    nc.scalar.activation(out=cat[0:D, :], in_=agg_ps[:],
                                         func=ACTF.Copy)
                    nc.scalar.activation(out=cat[D:2 * D, :],
                                         in_=xfm_s[:, b * 128:(b + 1) * 128],
                                         func=ACTF.Copy)
                    z_ps = psp.tile([D, 128], F32, tag="z")
                    nc.tensor.matmul(out=z_ps[:], lhsT=wc_s[:], rhs=cat[:],
                                     start=True, stop=True)
                    last_stage(b, z_ps, b_s)

            def leaky(dst_ap, z_ps, b_s):
                zb = wp.tile([D, 128], F32, tag="zb")
                nc.scalar.activation(out=zb[:], in_=z_ps[:],
                                     func=ACTF.Identity, bias=b_s[:, 0:1])
                t = wp.tile([D, 128], F32, tag="zt")
                nc.vector.tensor_scalar_mul(out=t[:], in0=zb[:],
                                            scalar1=cfg.LEAKY)
                nc.vector.tensor_tensor(out=dst_ap, in0=zb[:], in1=t[:],
                                        op=ALU.max)

            # ---------------- layer 1 ----------------
            def l1_tail(b, z_ps, b_s):
                hslice = h1T_s[:, b * 128:(b + 1) * 128]
                leaky(hslice, z_ps, b1_s)
                t_ps = psp.tile([128, D], F32, tag="tp")
                nc.tensor.transpose(out=t_ps[:], in_=hslice,
                                    identity=id64_s[:])
                h1nm = wp.tile([128, D], F32, tag="h1nm")
                nc.vector.tensor_copy(out=h1nm[:], in_=t_ps[:])
                nc.sync.dma_start(out=h1_local[b * 128:(b + 1) * 128, :],
                                  in_=h1nm[:])

            layer(idx1_s, x_d, N, w1c_s, b1_s, xT_s, l1_tail)

            nc.gpsimd.collective_compute(
                "AllGather",
                ALU.bypass,
                replica_groups=[list(range(CORES))],
                ins=[h1_local.ap()],
                outs=[h1_full.ap()],
            )

            # ---------------- layer 2 ----------------
            def l2_tail(b, z_ps, b_s):
                h2fm = wp.tile([D, 128], F32, tag="h2fm")
                leaky(h2fm[:], z_ps, b2_s)
                t_ps = psp.tile([128, D], F32, tag="tp")
                nc.tensor.transpose(out=t_ps[:], in_=h2fm[:],
                                    identity=id64_s[:])
                h2nm = wp.tile([128, D], F32, tag="h2nm")
                nc.vector.tensor_copy(out=h2nm[:], in_=t_ps[:])
                ph = wp.tile([128, 128], F32, tag="ph")
                nc.vector.tensor_scalar(
                    out=ph[:], in0=iota_s[:],
                    scalar1=gslot_s[:, b:b + 1], scalar2=None,
                    op0=ALU.is_equal)
                nc.tensor.matmul(out=pool_ps[:], lhsT=ph[:], rhs=h2nm[:],
                                 start=(b == 0), stop=(b == NB - 1))

            layer(idx2_s, h1_full, NPAD * CORES, w2c_s, b2_s, h1T_s, l2_tail)

            pool_s = wp.tile([128, D], F32, tag="pools")
            nc.scalar.activation(out=pool_s[:], in_=pool_ps[:], func=ACTF.Copy)
            nc.sync.dma_start(out=pool_d[:, :], in_=pool_s[:])

    nc.compile()
    return nc


# ---------------------------------------------------------------------------
# Entry point
# ---------------------------------------------------------------------------

_CACHE = {}


def _common_inputs(cfg, W1_root, W1_rel, W2_root, W2_rel, b1, b2):
    D = cfg.D
    return {
        "W1c": np.concatenate([W1_rel, W1_root], axis=0).astype(np.float32),
        "W2c": np.concatenate([W2_rel, W2_root], axis=0).astype(np.float32),
        "b1": np.ascontiguousarray(b1.reshape(D, 1).astype(np.float32)),
        "b2": np.ascontiguousarray(b2.reshape(D, 1).astype(np.float32)),
        "iota": np.broadcast_to(np.arange(128, dtype=np.float32),
                                (128, 128)).copy(),
        "id64": np.eye(D, dtype=np.float32),
    }


def run(cfg, inputs, trace=False):
    x = np.asarray(inputs["x_embeddings"], dtype=np.float32)
    in_maps, K_br, cs, g_base = preprocess(
        cfg, x, inputs["edge_index"], inputs["weights"], inputs["batch"])
    common = _common_inputs(cfg, inputs["W1_root"], inputs["W1_rel"],
                            inputs["W2_root"], inputs["W2_rel"],
                            inputs["b1"], inputs["b2"])
    for m in in_maps:
        m.update(common)

    key = (cfg.N, cfg.E, tuple(tuple(k) for k in K_br))
    if key not in _CACHE:
        _CACHE[key] = build_nc(cfg, K_br, cs)
    nc = _CACHE[key]

    res = run_bass_kernel_spmd(nc, in_maps, core_ids=list(range(cfg.CORES)),
                               trace=trace)

    batch = np.asarray(inputs["batch"], dtype=np.int64)
    counts = np.bincount(batch, minlength=cfg.G).astype(np.float32)
    pooled = np.zeros((cfg.G + 128, cfg.D), dtype=np.float32)
    for c in range(cfg.CORES):
        pooled[g_base[c]:g_base[c] + 128] += res.results[c]["pool"]
    pooled = pooled[:cfg.G] / np.maximum(counts, 1.0)[:, None]
    out = pooled @ np.asarray(inputs["Wl_root"], dtype=np.float32)
    out = out + np.asarray(inputs["bl"], dtype=np.float32)
    return out.astype(np.float32), res


def kernel(**inputs) -> np.ndarray:
    out, _ = run(REAL_CFG, inputs, trace=False)
    return out
